# revision 37
# baseline (speedup 1.0000x reference)
"""Trainium2 Bass kernel for HF-style causal self-attention (B=2, S=2048, D=2048,
H=16, head_dim=128), tensor-parallel over heads across 8 NeuronCores.

Sharding: core c computes heads {2c, 2c+1} for both batches (column-sharded
Wq/Wk/Wv). After per-head attention, an 8-rank AllToAll redistributes the
per-head context from head-sharding to token-sharding, and each core runs the
output projection (full Wo) for its 512-token slice. The host concatenates
the 8 token slices.

All matmul operands are fp16 (1 cycle/row on PE, half the DMA/SBUF/collective
bytes, FWL weight loads). PSUM accumulation is fp32; the output stays fp32.

Schedule (the point of this version): a soft pipeline. Projections run as
separate Q/K/V SWEEPS (one accumulator pair at a time, so projections +
attention PSUM pools coexist inside 8 banks), and the 32 attention units are
hand-placed into slots between sweeps so the softmax EXPs (ACT engine, the
attention bottleneck at ~110us total) hide under projection matmuls:

  A: per batch-0 group: Q/K sweeps + RoPE + V sweeps; V-only for batch-1
     groups; early batch-0 units interleave after their deps land.
  B: per batch-1 group: Q/K sweeps + RoPE, then the (b0 leftovers and) b1
     units of the matching group. The h0 AllToAll fires before the last h1
     unit; the h1 AllToAll right after it.
  C: o_proj pass 1 (head-0 features, under the h1 collective), pass 2.

Attention unit math: scores^T [k,q] chunks on PE, causally trimmed via a
runtime mask classification; exp on ACT (scale 1/sqrt(hd), no
max-subtraction -- unit-variance inputs); causal mask applied POST-exp as a
0/1 fp16 multiply; softmax denominators by DVE elementwise accumulation of
prob chunks + ONE ones-matmul per unit; normalization fused into the PSUM
evacuation (reciprocal+multiply on DVE).
"""

import math
import os

import numpy as np

# ---------------------------------------------------------------- constants
B, S, D = 2, 2048, 2048
H, HD = 16, 128
N_CORES = 8
LOCAL_H = H // N_CORES  # 2 heads per core
LOCAL_F = LOCAL_H * HD  # 256 local features
TOKS = B * S  # 4096 flat tokens (batch-major)
TG = 512  # token-group width (matmul moving dim)
NT = TOKS // TG  # 8 token groups
NB = S // 128  # 16 key blocks per batch
QB = TG // 128  # 4 query blocks per group
ROPE_BASE = 10000.0
SCALE = 1.0 / math.sqrt(HD)
SKIP_THRESH = -1e8  # mask block entirely masked if all values below this
LA = 3  # scores/exp chunks emitted ahead of PV (hides ACT latency from PE)

_NC_CACHE: dict = {}
last_exec_time_ns = None


# ---------------------------------------------------------------- host prep
def _rope_tables():
    inv_freq = 1.0 / (ROPE_BASE ** (np.arange(0, HD, 2, dtype=np.float64) / HD))
    t = np.arange(S, dtype=np.float64)
    freqs = np.outer(t, inv_freq)  # [S, HD/2]
    emb = np.concatenate([freqs, freqs], axis=-1)  # [S, HD]
    cos = np.cos(emb).T  # [HD, S]
    sin = np.sin(emb).T
    # rotate_half with the sign folded into a partition-aligned sin table:
    #   t2[0:64]   = s[64:128] * sinC[64:128]   (sinC[64:] = -sin[0:64])
    #   t2[64:128] = s[0:64]   * sinC[0:64]     (sinC[:64] = sin[64:128])
    # (DVE requires both SBUF inputs at the same base partition, so the
    # table rows live at the SOURCE partition of s, not the output's)
    half = HD // 2
    sinc = np.empty_like(sin)
    sinc[:half] = sin[half:]
    sinc[half:] = -sin[:half]
    return (np.ascontiguousarray(cos.astype(np.float16)),
            np.ascontiguousarray(sinc.astype(np.float16)))


def _classify_mask(mask2d):
    """Per 128x128 block of mask[q, k]: 0=all-zero, 1=needs mul, 2=fully masked."""
    nq, nk = S // 128, S // 128
    blocks = mask2d.reshape(nq, 128, nk, 128)
    mx = blocks.max(axis=(1, 3))
    mn = blocks.min(axis=(1, 3))
    cls = np.ones((nq, nk), dtype=np.int8)
    cls[(mx == 0.0) & (mn == 0.0)] = 0
    cls[mx < SKIP_THRESH] = 2
    return cls


def _build_plan(cls):
    """For each (query group g, key block j): None if skipped, else
    (comp_start_lqb, mul_start_lqb, mul_nblocks). The mul range spans the
    first to last local query block needing the 0/1 mask multiply."""
    plan = {}
    for g in range(4):
        for j in range(NB):
            lcls = [cls[4 * g + l, j] for l in range(QB)]
            if all(c == 2 for c in lcls):
                continue
            comp = min(l for l in range(QB) if lcls[l] != 2)
            muls = [l for l in range(comp, QB) if lcls[l] != 0]
            if muls:
                plan[(g, j)] = (comp, muls[0], muls[-1] - muls[0] + 1)
            else:
                plan[(g, j)] = (comp, 0, 0)
    return plan


def _plan_key(plan):
    return tuple(sorted((k, v) for k, v in plan.items()))


# ---------------------------------------------------------------- bass build
def _build(plan, n_mask_blocks, mask_idx):
    import concourse.bacc as bacc
    import concourse.mybir as mybir
    import concourse.tile as tile

    f32 = mybir.dt.float32
    mm = mybir.dt.float16

    nc = bacc.Bacc("TRN2", target_bir_lowering=False, debug=False,
                   num_devices=N_CORES)

    xT = nc.dram_tensor("xT", [D, TOKS], mm, kind="ExternalInput").ap()
    wq = nc.dram_tensor("wq", [D, LOCAL_F], mm, kind="ExternalInput").ap()
    wk = nc.dram_tensor("wk", [D, LOCAL_F], mm, kind="ExternalInput").ap()
    wv = nc.dram_tensor("wv", [D, LOCAL_F], mm, kind="ExternalInput").ap()
    wo = nc.dram_tensor("wo", [D, D], mm, kind="ExternalInput").ap()
    maskc = nc.dram_tensor("maskc", [max(n_mask_blocks, 1), 128, 512], mm,
                           kind="ExternalInput").ap()
    cosT = nc.dram_tensor("cosT", [HD, S], mm, kind="ExternalInput").ap()
    sinT = nc.dram_tensor("sinT", [HD, S], mm, kind="ExternalInput").ap()
    onesd = nc.dram_tensor("onesd", [128, 128], mm, kind="ExternalInput").ap()
    out = nc.dram_tensor("out", [TG, D], f32, kind="ExternalOutput").ap()

    with tile.TileContext(nc) as tc:
        with (
            tc.tile_pool(name="const", bufs=1) as constp,
            tc.tile_pool(name="dram", bufs=1, space="DRAM") as dram,
        ):
            cos_t = constp.tile([HD, S], mm, tag="cos")
            sin_t = constp.tile([HD, S], mm, tag="sin")
            ones_t = constp.tile([128, 128], mm, tag="ones")

            _attention_body(nc, tc, tile, mybir, mm, plan, mask_idx,
                            cos_t, sin_t, ones_t,
                            xT, wq, wk, wv, wo, out, dram,
                            maskc, cosT, sinT, onesd)

    nc.compile()
    return nc


def _attention_body(nc, tc, tile, mybir, mm, plan, mask_idx,
                    cos_t, sin_t, ones_t,
                    xT, wq, wk, wv, wo, out, dram,
                    maskc, cosT, sinT, onesd):
    f32 = mybir.dt.float32
    Exp = mybir.ActivationFunctionType.Exp
    ND = D // 128  # 16 contraction chunks
    half = HD // 2

    inb = [dram.tile([N_CORES, HD, TG], mm, name=f"inb{i}")
           for i in range(LOCAL_H)]
    outb = [dram.tile([N_CORES, HD, TG], mm, name=f"outb{i}")
            for i in range(LOCAL_H)]

    stack = [
        tc.tile_pool(name="qkres", bufs=LOCAL_H),
        tc.tile_pool(name="vres", bufs=LOCAL_H * 4 * NT),
        tc.tile_pool(name="wpool", bufs=3 * ND),
        tc.tile_pool(name="xpool", bufs=24),
        tc.tile_pool(name="ropes", bufs=3),
        tc.tile_pool(name="ropet", bufs=2),
        tc.tile_pool(name="ropeu", bufs=2),
        tc.tile_pool(name="maskp", bufs=1),
        tc.tile_pool(name="probs", bufs=8),
        tc.tile_pool(name="saccp", bufs=3),
        tc.tile_pool(name="recipp", bufs=2),
        tc.tile_pool(name="attnp", bufs=4),
        tc.tile_pool(name="wop", bufs=20),
    ]
    (qkres, vres, wpool, xpool, ropes, ropet1, ropet2, maskp, probs,
     saccp, recipp, attnp, wop) = [s.__enter__() for s in stack]

    # resident Q^T / K^T per local head [128, TOKS] (fp16)
    qt = [qkres.tile([HD, TOKS], mm, tag="qt", name=f"qt{i}") for i in range(LOCAL_H)]
    kt = [qkres.tile([HD, TOKS], mm, tag="kt", name=f"kt{i}") for i in range(LOCAL_H)]
    # resident V tiles [128 tok, HD] per (local head, flat token block):
    # per-head contiguous so the PV matmul's LDWEIGHTS is FWL-eligible
    v_sb = [[vres.tile([128, HD], mm, tag="v", name=f"v{h}_{i}")
             for i in range(TOKS // 128)] for h in range(LOCAL_H)]

    wq_t = [wpool.tile([128, LOCAL_F], mm, tag="w", name=f"wqt{i}") for i in range(ND)]
    wk_t = [wpool.tile([128, LOCAL_F], mm, tag="w", name=f"wkt{i}") for i in range(ND)]
    wv_t = [wpool.tile([128, LOCAL_F], mm, tag="w", name=f"wvt{i}") for i in range(ND)]

    # constants + mask strips on the scalar DMA queue: off the sync queue
    # (x/weight streaming) and off gpsimd (wo prefetch + collectives)
    nc.scalar.dma_start(out=cos_t, in_=cosT)
    nc.scalar.dma_start(out=sin_t, in_=sinT)
    nc.scalar.dma_start(out=ones_t, in_=onesd)

    # a tiny rendezvous at kernel start aligns the cores while phase A's
    # DMAs stream, so the real collectives later pay less skew
    barrier_in = dram.tile([N_CORES, 128, 2], mm, name="barrier_in")
    barrier_out = dram.tile([N_CORES, 128, 2], mm, name="barrier_out")
    nc.gpsimd.collective_compute(
        "AllToAll", mybir.AluOpType.bypass,
        replica_groups=[list(range(N_CORES))],
        ins=[barrier_in.opt()], outs=[barrier_out.opt()],
    )

    # mask strips ride the gpsimd queue behind the barrier: not needed
    # until the first attention unit (~190us in), and this keeps the
    # scalar queue free for the wv weight tiles at t=0
    mask_tiles = {}
    for key, (idx, nb) in mask_idx.items():
        mt = maskp.tile([128, 128 * nb], mm, tag=f"mb{idx}", name=f"mb{idx}")
        nc.gpsimd.dma_start(out=mt, in_=maskc[idx][:, :128 * nb])
        mask_tiles[key] = mt

    wo_t = {}
    # Load order = consumption order: evens (o_proj pass 1, head-0 features)
    # for all n-groups, then odds (pass 2). One shared tag, so slots released
    # by pass 1 recycle into odd-tile prefetch while pass 1 still runs.
    _wo_order = ([(n, f) for n in range(4) for f in range(0, ND, 2)]
                 + [(n, f) for n in range(4) for f in range(1, ND, 2)])

    def load_wo(k):
        n, f = _wo_order[k]
        w_t = wop.tile([128, TG], mm, tag="wo", name=f"wo{n}_{f}")
        nc.gpsimd.dma_start(out=w_t,
                            in_=wo[128 * f:128 * (f + 1),
                                   TG * n:TG * (n + 1)])
        wo_t[(n, f)] = w_t

    # prefetch only as many tiles as the pool has slots; the rest are
    # emitted inside the o_proj loop AFTER the collective triggers, so the
    # in-order gpsimd queue can't cycle
    for k in range(20):
        load_wo(k)
    _wo_next = [20]

    def load_wo_upto(k):
        while _wo_next[0] <= k:
            load_wo(_wo_next[0])
            _wo_next[0] += 1

    def rope_evac(ps, dst, csl, evac_eng):
        # RoPE on DVE: dst = s*cos + rot_half(s)*sin_signed
        s_t = ropes.tile([HD, TG], mm, tag="s")
        evac_eng(s_t, ps)  # fp32 PSUM -> fp16 SBUF
        t1 = ropet1.tile([HD, TG], mm, tag="t1")
        nc.vector.tensor_mul(t1, s_t, cos_t[:, csl])
        t2 = ropet2.tile([HD, TG], mm, tag="t2")
        nc.vector.tensor_mul(t2[:half], s_t[half:], sin_t[half:, csl])
        nc.vector.tensor_mul(t2[half:], s_t[:half], sin_t[:half, csl])
        nc.vector.tensor_add(dst, t1, t2)

    def qk_sweeps(t, xdma, evac_eng):
        """Q then K projection sweep for token group t + RoPE evacuation.
        Returns nothing; qt/kt columns for group t become valid."""
        tsl = slice(TG * t, TG * (t + 1))
        csl = slice(TG * (t % 4), TG * (t % 4 + 1))
        xts = []
        for wi, (w_t, wext, res) in enumerate(((wq_t, wq, qt),
                                               (wk_t, wk, kt))):
            acc = [psqk.tile([HD, TG], f32, tag="qk", name=f"qk{h}")
                   for h in range(LOCAL_H)]
            for j in range(ND):
                if t == 0:
                    nc.sync.dma_start(out=w_t[j],
                                      in_=wext[128 * j:128 * (j + 1), :])
                if wi == 0:
                    x_t = xpool.tile([128, TG], mm, tag="x")
                    xdma(out=x_t, in_=xT[128 * j:128 * (j + 1), tsl])
                    xts.append(x_t)
                for h in range(LOCAL_H):
                    hsl = slice(128 * h, 128 * (h + 1))
                    nc.tensor.matmul(acc[h], w_t[j][:, hsl], xts[j],
                                     start=(j == 0), stop=(j == ND - 1))
            for h in range(LOCAL_H):
                rope_evac(acc[h], res[h][:, tsl], csl, evac_eng)
        return xts

    def emit_scores(h, b, g, j, pe):
        comp, a0, nb = pe
        co = 128 * comp
        qsl = slice(2048 * b + TG * g, 2048 * b + TG * (g + 1))
        ksl = slice(2048 * b + 128 * j, 2048 * b + 128 * (j + 1))
        sc = pssc.tile([128, TG], f32, tag="sc", name="sc")
        nc.tensor.matmul(sc[:, co:], kt[h][:, ksl],
                         qt[h][:, qsl][:, co:],
                         start=True, stop=True)
        pt = probs.tile([128, TG], mm, tag="p", name="pt")
        nc.scalar.activation(pt[:, co:], sc[:, co:], Exp, scale=SCALE)
        if nb:
            # causal mask applied POST-exp as a 0/1 fp16 multiply (all-SBUF
            # 2-byte DVE op); raw scores are O(5) so exp never overflows
            mt = mask_tiles[(g, j)]
            q0 = 128 * a0
            nc.vector.tensor_mul(pt[:, q0:q0 + 128 * nb],
                                 pt[:, q0:q0 + 128 * nb], mt)
        return pt

    def emit_unit(h, b, g):
        """One attention unit: all key chunks of (head h, batch b, query
        group g), pipelined LA chunks deep, ending with the denominator
        reduce + normalize + bounce-buffer write."""
        chunks = [(j, plan[(g, j)]) for j in range(NB) if (g, j) in plan]
        n = len(chunks)
        pv_ps = pspv.tile([HD, TG], f32, tag="pv", name="pv")
        sa = saccp.tile([128, TG], mm, tag="sa", name="sacc")
        pts = []
        for i in range(n + LA):
            if i < n:
                j, pe = chunks[i]
                pts.append(emit_scores(h, b, g, j, pe))
            k = i - LA
            if k < 0 or k >= n:
                continue
            j, (comp, a0, nb) = chunks[k]
            pt = pts[k]
            co = 128 * comp
            first, last = k == 0, k == n - 1
            # softmax denominator partials: elementwise accumulate the prob
            # chunk on DVE (fp16 all-SBUF, 2x; gpsimd's software tensor_add
            # measured slower); partition-reduced once per unit by a single
            # ones-matmul below
            if first:
                nc.vector.tensor_copy(sa, pt)
            else:
                nc.vector.tensor_add(sa[:, co:], sa[:, co:], pt[:, co:])
            kb = 16 * b + j  # flat token block of this key chunk
            nc.tensor.matmul(pv_ps[:, co:], v_sb[h][kb], pt[:, co:],
                             start=first, stop=last)
        sum_ps = pssc.tile([128, TG], f32, tag="sc", name="sum")
        nc.tensor.matmul(sum_ps, ones_t, sa, start=True, stop=True)
        rec = recipp.tile([128, TG], f32, tag="rec")
        # ~18-bit reciprocal; sums are in [1, ~5e3] so no edge cases
        nc.vector.reciprocal_approx_fast(out=rec, in_=sum_ps)
        at = attnp.tile([HD, TG], mm, tag="at")
        nc.vector.tensor_mul(at, pv_ps, rec)
        nc.sync.dma_start(out=inb[h][4 * b + g], in_=at)

    # ---------------- phase A: b0 full QKV + b1 V, interleaved j-loop
    # (long Q/K matmuls + rotating PSUM banks keep every LDWEIGHTS hidden;
    # consecutive matmuls must not accumulate into the SAME bank -- the
    # fill cannot overlap the drain and each MM pays ~40ns)
    with tc.tile_pool(name="psA", bufs=8, space="PSUM") as psA:
        for t in range(NT):
            b0 = t < 4
            tsl = slice(TG * t, TG * (t + 1))
            csl = slice(TG * (t % 4), TG * (t % 4 + 1))
            if b0:
                acc = [psA.tile([HD, TG], f32, tag="qk", name=f"acc{i}")
                       for i in range(2 * LOCAL_H)]
            vacc = [psA.tile([128, TG], f32, tag="qk", name=f"vacc{i}")
                    for i in range(4)]
            for j in range(ND):
                if t == 0:
                    nc.sync.dma_start(out=wq_t[j], in_=wq[128 * j:128 * (j + 1), :])
                    # scalar queue (nearly empty after the consts): the sync
                    # queue at t=0 otherwise carries wq+wk+x = 4MB and the
                    # first ~8 j-chunks run DMA-starved
                    nc.scalar.dma_start(out=wk_t[j], in_=wk[128 * j:128 * (j + 1), :])
                x_t = xpool.tile([128, TG], mm, tag="x")
                nc.sync.dma_start(out=x_t, in_=xT[128 * j:128 * (j + 1), tsl])
                if t == 0:
                    # scalar HWDGE: off the sync queue carrying x/wq/wk
                    nc.scalar.dma_start(out=wv_t[j], in_=wv[128 * j:128 * (j + 1), :])
                if b0:
                    for wi, w_t in enumerate((wq_t, wk_t)):
                        for h in range(LOCAL_H):
                            hsl = slice(128 * h, 128 * (h + 1))
                            nc.tensor.matmul(acc[2 * wi + h], w_t[j][:, hsl],
                                             x_t,
                                             start=(j == 0), stop=(j == ND - 1))
                for m in range(4):
                    msl = slice(128 * m, 128 * (m + 1))
                    nc.tensor.matmul(vacc[m][:, :LOCAL_F], x_t[:, msl],
                                     wv_t[j],
                                     start=(j == 0), stop=(j == ND - 1))
            if b0:
                for wi, res in ((0, qt), (1, kt)):
                    for h in range(LOCAL_H):
                        rope_evac(acc[2 * wi + h], res[h][:, tsl], csl,
                                  nc.scalar.copy)
            for m in range(4):
                kb = 4 * t + m
                nc.scalar.copy(v_sb[0][kb], vacc[m][:, :HD])
                nc.vector.tensor_copy(v_sb[1][kb], vacc[m][:, HD:LOCAL_F])

    # PSUM for phases B/C: 3 (Q/K sweep accs) + 3 (scores+sum) + 2 (PV)
    psum_stack = [tc.tile_pool(name="psqk", bufs=3, space="PSUM"),
                  tc.tile_pool(name="pssc", bufs=3, space="PSUM"),
                  tc.tile_pool(name="pspv", bufs=2, space="PSUM")]
    psqk, pssc, pspv = [s.__enter__() for s in psum_stack]

    # ---------------- phase B: b1 Q/K sweeps + ALL attention units
    # (units slotted between sweeps so the EXP load -- the ACT engine is
    # the attention bottleneck -- hides under projection matmuls; b1 unit
    # (h,1,g) becomes ready after sweep t=4+g)
    B_UNITS = {4: [(0, 0, 0), (1, 0, 0), (0, 0, 1), (1, 0, 1)],
               5: [(0, 1, 0), (0, 0, 2), (1, 0, 2)],
               6: [(0, 1, 1), (0, 0, 3), (0, 1, 2)],
               7: [(0, 1, 3)]}
    for t in range(4, NT):
        # rope PSUM evacuation on DVE here: the ACT queue is deep in EXPs
        qk_sweeps(t, nc.scalar.dma_start, nc.vector.tensor_copy)
        for (h, b, g) in B_UNITS[t]:
            emit_unit(h, b, g)
    # AllToAll h0 fires as soon as the last h0 unit lands; the four h1
    # units held back here (~35us of PE+ACT work) cover its rendezvous
    # skew + transfer, so o_proj pass 1 starts with the data already home
    nc.gpsimd.collective_compute(
        "AllToAll", mybir.AluOpType.bypass,
        replica_groups=[list(range(N_CORES))],
        ins=[inb[0].opt()], outs=[outb[0].opt()],
    )
    for (h, b, g) in [(1, 0, 3), (1, 1, 0), (1, 1, 1), (1, 1, 2),
                      (1, 1, 3)]:
        emit_unit(h, b, g)
    nc.gpsimd.collective_compute(
        "AllToAll", mybir.AluOpType.bypass,
        replica_groups=[list(range(N_CORES))],
        ins=[inb[1].opt()], outs=[outb[1].opt()],
    )

    # close attention-era PSUM pools so o_proj gets its banks
    for s in reversed(psum_stack):
        s.__exit__(None, None, None)

    # ---------------- phase C: output projection for my 512-token slice
    with (
        tc.tile_pool(name="afull", bufs=D // 128) as afull,
        tc.tile_pool(name="outp", bufs=4) as outp,
        tc.tile_pool(name="psop", bufs=4, space="PSUM") as psop,
    ):
        af = [None] * (D // 128)

        def load_af(f):
            a_t = afull.tile([128, TG], mm, tag="af", name=f"af{f}")
            nc.sync.dma_start(out=a_t,
                               in_=outb[f % LOCAL_H][f // LOCAL_H])
            af[f] = a_t

        for f in range(0, D // 128, LOCAL_H):  # head-0 features for pass 1
            load_af(f)
        # pass 1: head-0 feature chunks only -- these land with the first
        # AllToAll, so this entire pass overlaps the second collective.
        # Partial sums are stashed in the dead qt tiles.
        evens = [f for f in range(ND) if f % LOCAL_H == 0]
        odds = [f for f in range(ND) if f % LOCAL_H != 0]
        for n in range(4):
            # emit the next block of wo loads (cycle-safe: we're past the
            # collective triggers in the gpsimd queue)
            load_wo_upto(8 * (n + 1) + 19)
            for m in range(4):
                p = 4 * n + m
                ps = psop.tile([128, TG], f32, tag="op", name="op1")
                for i, f in enumerate(evens):
                    nc.tensor.matmul(ps, af[f][:, 128 * m:128 * (m + 1)],
                                     wo_t[(n, f)],
                                     start=(i == 0), stop=(i == len(evens) - 1))
                # DVE, not ACT: the ACT queue is still draining the held-back
                # units' EXPs when pass 1 starts
                nc.vector.tensor_copy(
                    qt[p // 8][:, TG * (p % 8):TG * (p % 8 + 1)], ps)
        # pass 2: head-1 feature chunks + the stashed partial
        for f in range(1, D // 128, LOCAL_H):
            load_af(f)
        load_wo_upto(63)
        for n in range(4):
            nsl = slice(TG * n, TG * (n + 1))
            for m in range(4):
                p = 4 * n + m
                ps = psop.tile([128, TG], f32, tag="op", name="op2")
                for i, f in enumerate(odds):
                    nc.tensor.matmul(ps, af[f][:, 128 * m:128 * (m + 1)],
                                     wo_t[(n, f)],
                                     start=(i == 0), stop=(i == len(odds) - 1))
                o_t = outp.tile([128, TG], f32, tag="o")
                nc.vector.tensor_add(
                    o_t, ps,
                    qt[p // 8][:, TG * (p % 8):TG * (p % 8 + 1)])
                nc.sync.dma_start(out=out[128 * m:128 * (m + 1), nsl], in_=o_t)
    for s in reversed(stack):
        s.__exit__(None, None, None)


# ---------------------------------------------------------------- entry point
def kernel(x, mask, Wq, Wk, Wv, Wo):
    global last_exec_time_ns
    from concourse.bass_utils import run_bass_kernel_spmd

    x = np.asarray(x, dtype=np.float32)
    mask2d = np.ascontiguousarray(np.asarray(mask, dtype=np.float32)[0, 0])
    Wq = np.asarray(Wq, dtype=np.float32)
    Wk = np.asarray(Wk, dtype=np.float32)
    Wv = np.asarray(Wv, dtype=np.float32)
    Wo = np.ascontiguousarray(np.asarray(Wo, dtype=np.float32))

    # ---- host-side prep
    cls = _classify_mask(mask2d)
    plan = _build_plan(cls)
    mask01 = None
    mask_idx = {}
    strips = []
    for (g, j), (comp, a0, nb) in sorted(plan.items()):
        if nb == 0:
            continue
        if mask01 is None:
            mask01 = np.ascontiguousarray(mask2d.T)
        q0 = 512 * g + 128 * a0
        strip = np.ones((128, 512), dtype=np.float32)
        strip[:, :128 * nb] = (mask01[128 * j:128 * (j + 1),
                                      q0:q0 + 128 * nb] == 0.0)
        strips.append(strip)
        mask_idx[(g, j)] = (len(strips) - 1, nb)
    maskc = (np.stack(strips).astype(np.float16) if strips
             else np.zeros((1, 128, 512), dtype=np.float16))

    xTf = np.ascontiguousarray(x.reshape(TOKS, D).T.astype(np.float16))
    cosT, sinT = _rope_tables()

    key = _plan_key(plan)
    if key not in _NC_CACHE:
        _NC_CACHE[key] = _build(plan, len(strips), mask_idx)
    nc = _NC_CACHE[key]
    ones = np.ones((128, 128), dtype=np.float16)

    in_maps = []
    for c in range(N_CORES):
        fsl = slice(LOCAL_F * c, LOCAL_F * (c + 1))
        in_maps.append({
            "xT": xTf,
            "wq": np.ascontiguousarray(Wq[:, fsl].astype(np.float16)),
            "wk": np.ascontiguousarray(Wk[:, fsl].astype(np.float16)),
            "wv": np.ascontiguousarray(Wv[:, fsl].astype(np.float16)),
            "wo": Wo.astype(np.float16),
            "maskc": maskc,
            "cosT": cosT,
            "sinT": sinT,
            "onesd": ones,
        })

    trace = bool(os.environ.get("KERNEL_TRACE"))
    err = None
    for _ in range(3):
        try:
            res = run_bass_kernel_spmd(nc, in_maps,
                                       core_ids=list(range(N_CORES)),
                                       trace=trace)
            break
        except Exception as e:  # axon transport can be flaky; retry
            err = e
    else:
        raise err

    last_exec_time_ns = res.exec_time_ns
    out_flat = np.concatenate([res.results[c]["out"] for c in range(N_CORES)],
                              axis=0)
    return out_flat.reshape(B, S, D)


# revision 38
# speedup vs baseline: 1.0219x; 1.0219x over previous
"""Trainium2 Bass kernel for HF-style causal self-attention (B=2, S=2048, D=2048,
H=16, head_dim=128), tensor-parallel over heads across 8 NeuronCores.

Sharding: core c computes heads {2c, 2c+1} for both batches (column-sharded
Wq/Wk/Wv). After per-head attention, an 8-rank AllToAll redistributes the
per-head context from head-sharding to token-sharding, and each core runs the
output projection (full Wo) for its 512-token slice. The host concatenates
the 8 token slices.

All matmul operands are fp16 (1 cycle/row on PE, half the DMA/SBUF/collective
bytes, FWL weight loads). PSUM accumulation is fp32; the output stays fp32.

Schedule (the point of this version): a soft pipeline. Projections run as
separate Q/K/V SWEEPS (one accumulator pair at a time, so projections +
attention PSUM pools coexist inside 8 banks), and the 32 attention units are
hand-placed into slots between sweeps so the softmax EXPs (ACT engine, the
attention bottleneck at ~110us total) hide under projection matmuls:

  A: per batch-0 group: Q/K sweeps + RoPE + V sweeps; V-only for batch-1
     groups; early batch-0 units interleave after their deps land.
  B: per batch-1 group: Q/K sweeps + RoPE, then the (b0 leftovers and) b1
     units of the matching group. The h0 AllToAll fires before the last h1
     unit; the h1 AllToAll right after it.
  C: o_proj pass 1 (head-0 features, under the h1 collective), pass 2.

Attention unit math: scores^T [k,q] chunks on PE, causally trimmed via a
runtime mask classification; exp on ACT (scale 1/sqrt(hd), no
max-subtraction -- unit-variance inputs); causal mask applied POST-exp as a
0/1 fp16 multiply; softmax denominators by DVE elementwise accumulation of
prob chunks + ONE ones-matmul per unit; normalization fused into the PSUM
evacuation (reciprocal+multiply on DVE).
"""

import math
import os

import numpy as np

# ---------------------------------------------------------------- constants
B, S, D = 2, 2048, 2048
H, HD = 16, 128
N_CORES = 8
LOCAL_H = H // N_CORES  # 2 heads per core
LOCAL_F = LOCAL_H * HD  # 256 local features
TOKS = B * S  # 4096 flat tokens (batch-major)
TG = 512  # token-group width (matmul moving dim)
NT = TOKS // TG  # 8 token groups
NB = S // 128  # 16 key blocks per batch
QB = TG // 128  # 4 query blocks per group
ROPE_BASE = 10000.0
SCALE = 1.0 / math.sqrt(HD)
SKIP_THRESH = -1e8  # mask block entirely masked if all values below this
LA = 3  # scores/exp chunks emitted ahead of PV (hides ACT latency from PE)

_NC_CACHE: dict = {}
last_exec_time_ns = None


# ---------------------------------------------------------------- host prep
def _rope_tables():
    inv_freq = 1.0 / (ROPE_BASE ** (np.arange(0, HD, 2, dtype=np.float64) / HD))
    t = np.arange(S, dtype=np.float64)
    freqs = np.outer(t, inv_freq)  # [S, HD/2]
    emb = np.concatenate([freqs, freqs], axis=-1)  # [S, HD]
    cos = np.cos(emb).T  # [HD, S]
    sin = np.sin(emb).T
    # rotate_half with the sign folded into a partition-aligned sin table:
    #   t2[0:64]   = s[64:128] * sinC[64:128]   (sinC[64:] = -sin[0:64])
    #   t2[64:128] = s[0:64]   * sinC[0:64]     (sinC[:64] = sin[64:128])
    # (DVE requires both SBUF inputs at the same base partition, so the
    # table rows live at the SOURCE partition of s, not the output's)
    half = HD // 2
    sinc = np.empty_like(sin)
    sinc[:half] = sin[half:]
    sinc[half:] = -sin[:half]
    return (np.ascontiguousarray(cos.astype(np.float16)),
            np.ascontiguousarray(sinc.astype(np.float16)))


def _classify_mask(mask2d):
    """Per 128x128 block of mask[q, k]: 0=all-zero, 1=needs mul, 2=fully masked."""
    nq, nk = S // 128, S // 128
    blocks = mask2d.reshape(nq, 128, nk, 128)
    mx = blocks.max(axis=(1, 3))
    mn = blocks.min(axis=(1, 3))
    cls = np.ones((nq, nk), dtype=np.int8)
    cls[(mx == 0.0) & (mn == 0.0)] = 0
    cls[mx < SKIP_THRESH] = 2
    return cls


def _build_plan(cls):
    """For each (query group g, key block j): None if skipped, else
    (comp_start_lqb, mul_start_lqb, mul_nblocks). The mul range spans the
    first to last local query block needing the 0/1 mask multiply."""
    plan = {}
    for g in range(4):
        for j in range(NB):
            lcls = [cls[4 * g + l, j] for l in range(QB)]
            if all(c == 2 for c in lcls):
                continue
            comp = min(l for l in range(QB) if lcls[l] != 2)
            muls = [l for l in range(comp, QB) if lcls[l] != 0]
            if muls:
                plan[(g, j)] = (comp, muls[0], muls[-1] - muls[0] + 1)
            else:
                plan[(g, j)] = (comp, 0, 0)
    return plan


def _plan_key(plan):
    return tuple(sorted((k, v) for k, v in plan.items()))


# ---------------------------------------------------------------- bass build
def _build(plan, n_mask_blocks, mask_idx):
    import concourse.bacc as bacc
    import concourse.mybir as mybir
    import concourse.tile as tile

    f32 = mybir.dt.float32
    mm = mybir.dt.float16

    nc = bacc.Bacc("TRN2", target_bir_lowering=False, debug=False,
                   num_devices=N_CORES)

    xT = nc.dram_tensor("xT", [D, TOKS], mm, kind="ExternalInput").ap()
    wq = nc.dram_tensor("wq", [D, LOCAL_F], mm, kind="ExternalInput").ap()
    wk = nc.dram_tensor("wk", [D, LOCAL_F], mm, kind="ExternalInput").ap()
    wv = nc.dram_tensor("wv", [D, LOCAL_F], mm, kind="ExternalInput").ap()
    wo = nc.dram_tensor("wo", [D, D], mm, kind="ExternalInput").ap()
    maskc = nc.dram_tensor("maskc", [max(n_mask_blocks, 1), 128, 512], mm,
                           kind="ExternalInput").ap()
    cosT = nc.dram_tensor("cosT", [HD, S], mm, kind="ExternalInput").ap()
    sinT = nc.dram_tensor("sinT", [HD, S], mm, kind="ExternalInput").ap()
    onesd = nc.dram_tensor("onesd", [128, 128], mm, kind="ExternalInput").ap()
    out = nc.dram_tensor("out", [TG, D], f32, kind="ExternalOutput").ap()

    with tile.TileContext(nc) as tc:
        with (
            tc.tile_pool(name="const", bufs=1) as constp,
            tc.tile_pool(name="dram", bufs=1, space="DRAM") as dram,
        ):
            cos_t = constp.tile([HD, S], mm, tag="cos")
            sin_t = constp.tile([HD, S], mm, tag="sin")
            ones_t = constp.tile([128, 128], mm, tag="ones")

            _attention_body(nc, tc, tile, mybir, mm, plan, mask_idx,
                            cos_t, sin_t, ones_t,
                            xT, wq, wk, wv, wo, out, dram,
                            maskc, cosT, sinT, onesd)

    nc.compile()
    return nc


def _attention_body(nc, tc, tile, mybir, mm, plan, mask_idx,
                    cos_t, sin_t, ones_t,
                    xT, wq, wk, wv, wo, out, dram,
                    maskc, cosT, sinT, onesd):
    f32 = mybir.dt.float32
    Exp = mybir.ActivationFunctionType.Exp
    ND = D // 128  # 16 contraction chunks
    half = HD // 2

    inb = [dram.tile([N_CORES, HD, TG], mm, name=f"inb{i}")
           for i in range(LOCAL_H)]
    outb = [dram.tile([N_CORES, HD, TG], mm, name=f"outb{i}")
            for i in range(LOCAL_H)]

    stack = [
        tc.tile_pool(name="qkres", bufs=LOCAL_H),
        tc.tile_pool(name="vres", bufs=LOCAL_H * 4 * NT),
        tc.tile_pool(name="wpool", bufs=3 * ND),
        tc.tile_pool(name="xpool", bufs=24),
        tc.tile_pool(name="ropes", bufs=3),
        tc.tile_pool(name="ropet", bufs=2),
        tc.tile_pool(name="ropeu", bufs=2),
        tc.tile_pool(name="maskp", bufs=1),
        tc.tile_pool(name="probs", bufs=6),
        tc.tile_pool(name="saccp", bufs=3),
        tc.tile_pool(name="recipp", bufs=2),
        tc.tile_pool(name="attnp", bufs=4),
        tc.tile_pool(name="wop", bufs=20),
    ]
    (qkres, vres, wpool, xpool, ropes, ropet1, ropet2, maskp, probs,
     saccp, recipp, attnp, wop) = [s.__enter__() for s in stack]

    # resident Q^T / K^T per local head [128, TOKS] (fp16)
    qt = [qkres.tile([HD, TOKS], mm, tag="qt", name=f"qt{i}") for i in range(LOCAL_H)]
    kt = [qkres.tile([HD, TOKS], mm, tag="kt", name=f"kt{i}") for i in range(LOCAL_H)]
    # resident V tiles [128 tok, HD] per (local head, flat token block):
    # per-head contiguous so the PV matmul's LDWEIGHTS is FWL-eligible
    v_sb = [[vres.tile([128, HD], mm, tag="v", name=f"v{h}_{i}")
             for i in range(TOKS // 128)] for h in range(LOCAL_H)]

    wq_t = [wpool.tile([128, LOCAL_F], mm, tag="w", name=f"wqt{i}") for i in range(ND)]
    wk_t = [wpool.tile([128, LOCAL_F], mm, tag="w", name=f"wkt{i}") for i in range(ND)]
    wv_t = [wpool.tile([128, LOCAL_F], mm, tag="w", name=f"wvt{i}") for i in range(ND)]

    # constants + mask strips on the scalar DMA queue: off the sync queue
    # (x/weight streaming) and off gpsimd (wo prefetch + collectives)
    nc.scalar.dma_start(out=cos_t, in_=cosT)
    nc.scalar.dma_start(out=sin_t, in_=sinT)
    nc.scalar.dma_start(out=ones_t, in_=onesd)

    # a tiny rendezvous at kernel start aligns the cores while phase A's
    # DMAs stream, so the real collectives later pay less skew
    barrier_in = dram.tile([N_CORES, 128, 2], mm, name="barrier_in")
    barrier_out = dram.tile([N_CORES, 128, 2], mm, name="barrier_out")
    nc.gpsimd.collective_compute(
        "AllToAll", mybir.AluOpType.bypass,
        replica_groups=[list(range(N_CORES))],
        ins=[barrier_in.opt()], outs=[barrier_out.opt()],
    )

    # mask strips ride the gpsimd queue behind the barrier: not needed
    # until the first attention unit (~190us in), and this keeps the
    # scalar queue free for the wv weight tiles at t=0
    mask_tiles = {}
    for key, (idx, nb) in mask_idx.items():
        mt = maskp.tile([128, 128 * nb], mm, tag=f"mb{idx}", name=f"mb{idx}")
        nc.gpsimd.dma_start(out=mt, in_=maskc[idx][:, :128 * nb])
        mask_tiles[key] = mt

    wo_t = {}
    # Load order = consumption order: evens (o_proj pass 1, head-0 features)
    # for all n-groups, then odds (pass 2). One shared tag, so slots released
    # by pass 1 recycle into odd-tile prefetch while pass 1 still runs.
    _wo_order = ([(n, f) for n in range(4) for f in range(0, ND, 2)]
                 + [(n, f) for n in range(4) for f in range(1, ND, 2)])

    def load_wo(k):
        n, f = _wo_order[k]
        w_t = wop.tile([128, TG], mm, tag="wo", name=f"wo{n}_{f}")
        nc.gpsimd.dma_start(out=w_t,
                            in_=wo[128 * f:128 * (f + 1),
                                   TG * n:TG * (n + 1)])
        wo_t[(n, f)] = w_t

    # prefetch only as many tiles as the pool has slots; the rest are
    # emitted inside the o_proj loop AFTER the collective triggers, so the
    # in-order gpsimd queue can't cycle
    for k in range(20):
        load_wo(k)
    _wo_next = [20]

    def load_wo_upto(k):
        while _wo_next[0] <= k:
            load_wo(_wo_next[0])
            _wo_next[0] += 1

    def rope_evac(ps, dst, csl, evac_eng):
        # RoPE on DVE: dst = s*cos + rot_half(s)*sin_signed
        s_t = ropes.tile([HD, TG], mm, tag="s")
        evac_eng(s_t, ps)  # fp32 PSUM -> fp16 SBUF
        t1 = ropet1.tile([HD, TG], mm, tag="t1")
        nc.vector.tensor_mul(t1, s_t, cos_t[:, csl])
        t2 = ropet2.tile([HD, TG], mm, tag="t2")
        nc.vector.tensor_mul(t2[:half], s_t[half:], sin_t[half:, csl])
        nc.vector.tensor_mul(t2[half:], s_t[:half], sin_t[:half, csl])
        nc.vector.tensor_add(dst, t1, t2)

    def qk_sweeps(t, xdma, evac_eng):
        """Q then K projection sweep for token group t + RoPE evacuation.
        Returns nothing; qt/kt columns for group t become valid."""
        tsl = slice(TG * t, TG * (t + 1))
        csl = slice(TG * (t % 4), TG * (t % 4 + 1))
        xts = []
        for wi, (w_t, wext, res) in enumerate(((wq_t, wq, qt),
                                               (wk_t, wk, kt))):
            acc = [psqk.tile([HD, TG], f32, tag="qk", name=f"qk{h}")
                   for h in range(LOCAL_H)]
            for j in range(ND):
                if t == 0:
                    nc.sync.dma_start(out=w_t[j],
                                      in_=wext[128 * j:128 * (j + 1), :])
                if wi == 0:
                    x_t = xpool.tile([128, TG], mm, tag="x")
                    xdma(out=x_t, in_=xT[128 * j:128 * (j + 1), tsl])
                    xts.append(x_t)
                for h in range(LOCAL_H):
                    hsl = slice(128 * h, 128 * (h + 1))
                    nc.tensor.matmul(acc[h], w_t[j][:, hsl], xts[j],
                                     start=(j == 0), stop=(j == ND - 1))
            for h in range(LOCAL_H):
                rope_evac(acc[h], res[h][:, tsl], csl, evac_eng)
        return xts

    def emit_scores(h, b, g, j, pe):
        comp, a0, nb = pe
        co = 128 * comp
        qsl = slice(2048 * b + TG * g, 2048 * b + TG * (g + 1))
        ksl = slice(2048 * b + 128 * j, 2048 * b + 128 * (j + 1))
        sc = pssc.tile([128, TG], f32, tag="sc", name="sc")
        nc.tensor.matmul(sc[:, co:], kt[h][:, ksl],
                         qt[h][:, qsl][:, co:],
                         start=True, stop=True)
        pt = probs.tile([128, TG], mm, tag="p", name="pt")
        nc.scalar.activation(pt[:, co:], sc[:, co:], Exp, scale=SCALE)
        if nb:
            # causal mask applied POST-exp as a 0/1 fp16 multiply (all-SBUF
            # 2-byte DVE op); raw scores are O(5) so exp never overflows
            mt = mask_tiles[(g, j)]
            q0 = 128 * a0
            nc.vector.tensor_mul(pt[:, q0:q0 + 128 * nb],
                                 pt[:, q0:q0 + 128 * nb], mt)
        return pt

    def emit_unit(h, b, g):
        """One attention unit: all key chunks of (head h, batch b, query
        group g), pipelined LA chunks deep, ending with the denominator
        reduce + normalize + bounce-buffer write."""
        chunks = [(j, plan[(g, j)]) for j in range(NB) if (g, j) in plan]
        n = len(chunks)
        pv_ps = pspv.tile([HD, TG], f32, tag="pv", name="pv")
        sa = saccp.tile([128, TG], mm, tag="sa", name="sacc")
        pts = []
        for i in range(n + LA):
            if i < n:
                j, pe = chunks[i]
                pts.append(emit_scores(h, b, g, j, pe))
            k = i - LA
            if k < 0 or k >= n:
                continue
            j, (comp, a0, nb) = chunks[k]
            pt = pts[k]
            co = 128 * comp
            first, last = k == 0, k == n - 1
            # softmax denominator partials: elementwise accumulate the prob
            # chunk on DVE (fp16 all-SBUF, 2x; gpsimd's software tensor_add
            # measured slower); partition-reduced once per unit by a single
            # ones-matmul below
            if first:
                nc.vector.tensor_copy(sa, pt)
            else:
                nc.vector.tensor_add(sa[:, co:], sa[:, co:], pt[:, co:])
            kb = 16 * b + j  # flat token block of this key chunk
            nc.tensor.matmul(pv_ps[:, co:], v_sb[h][kb], pt[:, co:],
                             start=first, stop=last)
        sum_ps = pssc.tile([128, TG], f32, tag="sc", name="sum")
        nc.tensor.matmul(sum_ps, ones_t, sa, start=True, stop=True)
        rec = recipp.tile([128, TG], f32, tag="rec")
        # ~18-bit reciprocal; sums are in [1, ~5e3] so no edge cases
        nc.vector.reciprocal_approx_fast(out=rec, in_=sum_ps)
        at = attnp.tile([HD, TG], mm, tag="at")
        nc.vector.tensor_mul(at, pv_ps, rec)
        nc.sync.dma_start(out=inb[h][4 * b + g], in_=at)

    # ---------------- phase A: b0 full QKV + b1 V, interleaved j-loop
    # (long Q/K matmuls + rotating PSUM banks keep every LDWEIGHTS hidden;
    # consecutive matmuls must not accumulate into the SAME bank -- the
    # fill cannot overlap the drain and each MM pays ~40ns)
    with tc.tile_pool(name="psA", bufs=8, space="PSUM") as psA:
        for t in range(NT):
            b0 = t < 4
            tsl = slice(TG * t, TG * (t + 1))
            csl = slice(TG * (t % 4), TG * (t % 4 + 1))
            if b0:
                acc = [psA.tile([HD, TG], f32, tag="qk", name=f"acc{i}")
                       for i in range(2 * LOCAL_H)]
            vacc = [psA.tile([128, TG], f32, tag="qk", name=f"vacc{i}")
                    for i in range(4)]
            for j in range(ND):
                if t == 0:
                    nc.sync.dma_start(out=wq_t[j], in_=wq[128 * j:128 * (j + 1), :])
                    nc.sync.dma_start(out=wk_t[j], in_=wk[128 * j:128 * (j + 1), :])
                x_t = xpool.tile([128, TG], mm, tag="x")
                nc.sync.dma_start(out=x_t, in_=xT[128 * j:128 * (j + 1), tsl])
                if t == 0:
                    # scalar HWDGE: off the sync queue carrying x/wq/wk
                    nc.scalar.dma_start(out=wv_t[j], in_=wv[128 * j:128 * (j + 1), :])
                if b0:
                    for wi, w_t in enumerate((wq_t, wk_t)):
                        for h in range(LOCAL_H):
                            hsl = slice(128 * h, 128 * (h + 1))
                            nc.tensor.matmul(acc[2 * wi + h], w_t[j][:, hsl],
                                             x_t,
                                             start=(j == 0), stop=(j == ND - 1))
                for m in range(4):
                    msl = slice(128 * m, 128 * (m + 1))
                    nc.tensor.matmul(vacc[m][:, :LOCAL_F], x_t[:, msl],
                                     wv_t[j],
                                     start=(j == 0), stop=(j == ND - 1))
            if b0:
                for wi, res in ((0, qt), (1, kt)):
                    for h in range(LOCAL_H):
                        rope_evac(acc[2 * wi + h], res[h][:, tsl], csl,
                                  nc.scalar.copy)
            for m in range(4):
                kb = 4 * t + m
                nc.scalar.copy(v_sb[0][kb], vacc[m][:, :HD])
                nc.vector.tensor_copy(v_sb[1][kb], vacc[m][:, HD:LOCAL_F])

    # PSUM for phases B/C: 3 (Q/K sweep accs) + 3 (scores+sum) + 2 (PV)
    psum_stack = [tc.tile_pool(name="psqk", bufs=3, space="PSUM"),
                  tc.tile_pool(name="pssc", bufs=3, space="PSUM"),
                  tc.tile_pool(name="pspv", bufs=2, space="PSUM")]
    psqk, pssc, pspv = [s.__enter__() for s in psum_stack]

    # ---------------- phase B: b1 Q/K sweeps + ALL attention units
    # (units slotted between sweeps so the EXP load -- the ACT engine is
    # the attention bottleneck -- hides under projection matmuls; b1 unit
    # (h,1,g) becomes ready after sweep t=4+g)
    B_UNITS = {4: [(0, 0, 0), (1, 0, 0), (0, 0, 1), (1, 0, 1)],
               5: [(0, 1, 0), (0, 0, 2), (1, 0, 2)],
               6: [(0, 1, 1), (0, 0, 3), (0, 1, 2)],
               7: [(0, 1, 3)]}
    for t in range(4, NT):
        # rope PSUM evacuation on DVE here: the ACT queue is deep in EXPs
        qk_sweeps(t, nc.scalar.dma_start, nc.vector.tensor_copy)
        for (h, b, g) in B_UNITS[t]:
            emit_unit(h, b, g)
    # AllToAll h0 fires as soon as the last h0 unit lands; the four h1
    # units held back here (~35us of PE+ACT work) cover its rendezvous
    # skew + transfer, so o_proj pass 1 starts with the data already home
    nc.gpsimd.collective_compute(
        "AllToAll", mybir.AluOpType.bypass,
        replica_groups=[list(range(N_CORES))],
        ins=[inb[0].opt()], outs=[outb[0].opt()],
    )
    for (h, b, g) in [(1, 0, 3), (1, 1, 0), (1, 1, 1), (1, 1, 2),
                      (1, 1, 3)]:
        emit_unit(h, b, g)
    nc.gpsimd.collective_compute(
        "AllToAll", mybir.AluOpType.bypass,
        replica_groups=[list(range(N_CORES))],
        ins=[inb[1].opt()], outs=[outb[1].opt()],
    )

    # close attention-era PSUM pools so o_proj gets its banks
    for s in reversed(psum_stack):
        s.__exit__(None, None, None)

    # ---------------- phase C: output projection for my 512-token slice
    with (
        tc.tile_pool(name="afull", bufs=D // 128) as afull,
        tc.tile_pool(name="outp", bufs=4) as outp,
        tc.tile_pool(name="psop", bufs=4, space="PSUM") as psop,
    ):
        af = [None] * (D // 128)

        def load_af(f):
            a_t = afull.tile([128, TG], mm, tag="af", name=f"af{f}")
            nc.sync.dma_start(out=a_t,
                               in_=outb[f % LOCAL_H][f // LOCAL_H])
            af[f] = a_t

        for f in range(0, D // 128, LOCAL_H):  # head-0 features for pass 1
            load_af(f)
        # pass 1: head-0 feature chunks only -- these land with the first
        # AllToAll, so this entire pass overlaps the second collective.
        # Partial sums are stashed in the dead qt tiles.
        evens = [f for f in range(ND) if f % LOCAL_H == 0]
        odds = [f for f in range(ND) if f % LOCAL_H != 0]
        for n in range(4):
            # emit the next block of wo loads (cycle-safe: we're past the
            # collective triggers in the gpsimd queue)
            load_wo_upto(8 * (n + 1) + 19)
            for m in range(4):
                p = 4 * n + m
                ps = psop.tile([128, TG], f32, tag="op", name="op1")
                for i, f in enumerate(evens):
                    nc.tensor.matmul(ps, af[f][:, 128 * m:128 * (m + 1)],
                                     wo_t[(n, f)],
                                     start=(i == 0), stop=(i == len(evens) - 1))
                # DVE, not ACT: the ACT queue is still draining the held-back
                # units' EXPs when pass 1 starts
                nc.vector.tensor_copy(
                    qt[p // 8][:, TG * (p % 8):TG * (p % 8 + 1)], ps)
        # pass 2: head-1 feature chunks + the stashed partial
        for f in range(1, D // 128, LOCAL_H):
            load_af(f)
        load_wo_upto(63)
        for n in range(4):
            nsl = slice(TG * n, TG * (n + 1))
            for m in range(4):
                p = 4 * n + m
                ps = psop.tile([128, TG], f32, tag="op", name="op2")
                for i, f in enumerate(odds):
                    nc.tensor.matmul(ps, af[f][:, 128 * m:128 * (m + 1)],
                                     wo_t[(n, f)],
                                     start=(i == 0), stop=(i == len(odds) - 1))
                o_t = outp.tile([128, TG], f32, tag="o")
                nc.vector.tensor_add(
                    o_t, ps,
                    qt[p // 8][:, TG * (p % 8):TG * (p % 8 + 1)])
                nc.sync.dma_start(out=out[128 * m:128 * (m + 1), nsl], in_=o_t)
    for s in reversed(stack):
        s.__exit__(None, None, None)


# ---------------------------------------------------------------- entry point
def kernel(x, mask, Wq, Wk, Wv, Wo):
    global last_exec_time_ns
    from concourse.bass_utils import run_bass_kernel_spmd

    x = np.asarray(x, dtype=np.float32)
    mask2d = np.ascontiguousarray(np.asarray(mask, dtype=np.float32)[0, 0])
    Wq = np.asarray(Wq, dtype=np.float32)
    Wk = np.asarray(Wk, dtype=np.float32)
    Wv = np.asarray(Wv, dtype=np.float32)
    Wo = np.ascontiguousarray(np.asarray(Wo, dtype=np.float32))

    # ---- host-side prep
    cls = _classify_mask(mask2d)
    plan = _build_plan(cls)
    mask01 = None
    mask_idx = {}
    strips = []
    for (g, j), (comp, a0, nb) in sorted(plan.items()):
        if nb == 0:
            continue
        if mask01 is None:
            mask01 = np.ascontiguousarray(mask2d.T)
        q0 = 512 * g + 128 * a0
        strip = np.ones((128, 512), dtype=np.float32)
        strip[:, :128 * nb] = (mask01[128 * j:128 * (j + 1),
                                      q0:q0 + 128 * nb] == 0.0)
        strips.append(strip)
        mask_idx[(g, j)] = (len(strips) - 1, nb)
    maskc = (np.stack(strips).astype(np.float16) if strips
             else np.zeros((1, 128, 512), dtype=np.float16))

    xTf = np.ascontiguousarray(x.reshape(TOKS, D).T.astype(np.float16))
    cosT, sinT = _rope_tables()

    key = _plan_key(plan)
    if key not in _NC_CACHE:
        _NC_CACHE[key] = _build(plan, len(strips), mask_idx)
    nc = _NC_CACHE[key]
    ones = np.ones((128, 128), dtype=np.float16)

    in_maps = []
    for c in range(N_CORES):
        fsl = slice(LOCAL_F * c, LOCAL_F * (c + 1))
        in_maps.append({
            "xT": xTf,
            "wq": np.ascontiguousarray(Wq[:, fsl].astype(np.float16)),
            "wk": np.ascontiguousarray(Wk[:, fsl].astype(np.float16)),
            "wv": np.ascontiguousarray(Wv[:, fsl].astype(np.float16)),
            "wo": Wo.astype(np.float16),
            "maskc": maskc,
            "cosT": cosT,
            "sinT": sinT,
            "onesd": ones,
        })

    trace = bool(os.environ.get("KERNEL_TRACE"))
    err = None
    for _ in range(3):
        try:
            res = run_bass_kernel_spmd(nc, in_maps,
                                       core_ids=list(range(N_CORES)),
                                       trace=trace)
            break
        except Exception as e:  # axon transport can be flaky; retry
            err = e
    else:
        raise err

    last_exec_time_ns = res.exec_time_ns
    out_flat = np.concatenate([res.results[c]["out"] for c in range(N_CORES)],
                              axis=0)
    return out_flat.reshape(B, S, D)
